# revision 36
# baseline (speedup 1.0000x reference)
"""Trainium2 Bass kernel for nn_Attention_16707422781936.

Data-parallel over batch: B=8 -> one batch element per NeuronCore (8 cores).
Per core: qkv 1x1-conv GEMM, 8-head softmax attention over N=1600 tokens,
proj GEMM, depthwise 3x3 positional-encoding conv, summed output.

Optimizations over the v1 kernel:
 - softmax exp split across ScalarE (exact, fp8 out) and DVE/Pool
   (Schraudolph int8 bit-trick producing fp8e4m3 directly)
 - AV matmul in fp8e4 DoubleRow mode (256-deep contraction per instr)
 - scores matmuls cycle PE row-bands (mi parity) -> 4-band concurrency
 - v computed once (channel-major); token-major copy via DMA transpose
 - v bias folded into bpe (host) so v is bias-free on device
 - dwconv in bf16 on DVE (2x mode); evacuations moved to Pool engine
 - softmax normalize uses a 0-stride partition-broadcast AP (no
   gpsimd partition_broadcast op)
"""
import math
import sys

sys.path.insert(0, "/opt/trn_rl_repo")

import ml_dtypes
import numpy as np

import concourse.bass as bass
import concourse.mybir as mybir
import concourse.tile as tile
from concourse import bacc
from concourse.bass_utils import run_bass_kernel_spmd

F32 = mybir.dt.float32
F32R = mybir.dt.float32r
BF16 = mybir.dt.bfloat16
FP8 = mybir.dt.float8e4
I8 = mybir.dt.int8
ALU = mybir.AluOpType
EXP = mybir.ActivationFunctionType.Exp
DRM = mybir.MatmulPerfMode.DoubleRow

C = 512          # channels
N = 1600         # tokens (40*40)
H = W = 40
NH = 8           # heads
KD = 32          # key dim
HD = 64          # head dim (v)
SCALE = KD ** -0.5

# Schraudolph fast-exp constants for fp8e4m3 bit pattern:
# bits = round(s * SCALE * 8/ln2 + (7*8 - C_ADJ)); bits viewed as fp8e4m3
# approximate exp(s * SCALE).
SCH_C = -0.1
SCH_A = 8.0 * SCALE / math.log(2.0)
SCH_B = 7.0 * 8.0 - SCH_C

# n blocks (psum-bank sized) and m tiles (partition sized)
NB = [(0, 512), (512, 512), (1024, 512), (1536, 64)]
MT = [(i * 128, min(128, N - i * 128)) for i in range(13)]
# exp chunk engine per mi: S=ScalarE exact, D=DVE schraudolph
# (Pool/gpsimd cannot read PSUM, so it gets no exp chunks)
EXP_ENG = ['S', 'D', 'S', 'D', 'S', 'D', 'S', 'D', 'S', 'D', 'S', 'D', 'S']
VT_W = 72        # padded vt row width (65 used) so DR k-pair stride % 16 == 0
NORM_FIRE_MI = 4   # sweep step at which the previous sweep's deferred normalize fires

_CACHE = {}


def build():
    nc = bacc.Bacc("TRN2", target_bir_lowering=False, debug=False,
                   enable_asserts=False)

    x_d = nc.dram_tensor("x", [C, N], BF16, kind="ExternalInput").ap()
    wqkvt_d = nc.dram_tensor("wqkvt", [128, 4 * 1024], BF16, kind="ExternalInput").ap()
    wprojt_d = nc.dram_tensor("wprojt", [128, 4 * 512], BF16, kind="ExternalInput").ap()
    bqrow_d = nc.dram_tensor("bqrow", [1, 256], BF16, kind="ExternalInput").ap()
    bvrow_d = nc.dram_tensor("bvrow", [1, 512], BF16, kind="ExternalInput").ap()
    bproj_d = nc.dram_tensor("bproj", [1, 512], BF16, kind="ExternalInput").ap()
    bpe_d = nc.dram_tensor("bpe", [128, 4], F32, kind="ExternalInput").ap()
    wpe_d = nc.dram_tensor("wpe", [128, 36], F32, kind="ExternalInput").ap()
    ones_d = nc.dram_tensor("ones8", [128, NH], FP8, kind="ExternalInput").ap()
    out_d = nc.dram_tensor("out", [C, N], F32, kind="ExternalOutput").ap()

    with nc.allow_low_precision(reason="bf16/fp8 attention within 2e-2 tolerance"), \
         tile.TileContext(nc) as tc:
        with tc.tile_pool(name="persist", bufs=1) as per:
            x_sb = per.tile([128, 4, N], BF16, tag="x")
            qk_sb = per.tile([128, 4, N], BF16, tag="qk")   # q(h0-3),q(h4-7),k(h0-3),k(h4-7)
            v_sb = per.tile([128, 4, N], BF16, tag="v")     # v channel-major, no bias
            vtb_sb = per.tile([128, 13, 128], BF16, tag="vtb")  # DMA-transpose staging
            vt_sb = per.tile([128, 13, NH, VT_W], FP8, tag="vt")  # v^T fp8 + ones col
            attn_sb = per.tile([128, 4, N], BF16, tag="attn")
            pe_sb = per.tile([128, 4, H, W], BF16, tag="pe")
            wqkvt_sb = per.tile([128, 4, 1024], BF16, tag="wqkvt")
            wprojt_sb = per.tile([128, 4, 512], BF16, tag="wprojt")
            bqrow_sb = per.tile([1, 256], BF16, tag="bqrow")
            bvrow_sb = per.tile([1, 512], BF16, tag="bvrow")
            bproj_sb = per.tile([1, 512], BF16, tag="bproj")
            bpe_sb = per.tile([128, 4], F32, tag="bpe")
            wpe_sb = per.tile([128, 36], F32, tag="wpe")

            nc.sync.dma_start(wqkvt_sb[:], wqkvt_d.rearrange("p (t o) -> p t o", t=4))
            nc.sync.dma_start(wprojt_sb[:], wprojt_d.rearrange("p (t o) -> p t o", t=4))
            nc.sync.dma_start(bqrow_sb[:], bqrow_d)
            nc.sync.dma_start(bvrow_sb[:], bvrow_d)
            nc.sync.dma_start(bproj_sb[:], bproj_d)
            nc.sync.dma_start(bpe_sb[:], bpe_d)
            nc.sync.dma_start(wpe_sb[:], wpe_d)
            x_dr = x_d.rearrange("(t p) n -> p t n", p=128)
            for kt in range(4):
                nc.sync.dma_start(x_sb[:, kt], x_dr[:, kt])
            for mi in range(13):
                nc.sync.dma_start(vt_sb[:, mi, :, HD:HD + 1],
                                  ones_d.rearrange("p (h o) -> p h o", o=1))
            # [1, N] bf16 ones row: K=1 matmul adds the q bias into the
            # qkv PSUM accumulation (k bias cancels in softmax; v bias is
            # folded into bproj/bpe on the host)
            onesn_sb = per.tile([1, N], BF16, tag="onesn")
            nc.gpsimd.memset(onesn_sb[:], 1.0)

            v4 = v_sb[:].rearrange("p t (h w) -> p t h w", h=H)
            pe3 = pe_sb[:].rearrange("p t h w -> p t (h w)")
            out_dr = out_d.rearrange("(t p) n -> p t n", p=128)

            # ---------- building blocks ----------
            def qk_group(pool, mt, n0, nw):
                is_q = mt < 2
                qp = pool.tile([128, 512], F32, tag="av", name="qg")
                for kt in range(4):
                    nc.tensor.matmul(
                        qp[0:128, 0:nw],
                        wqkvt_sb[:, kt, mt * 128:(mt + 1) * 128],
                        x_sb[:, kt, n0:n0 + nw],
                        start=(kt == 0), stop=(kt == 3 and not is_q))
                if is_q:
                    nc.tensor.matmul(
                        qp[0:128, 0:nw],
                        bqrow_sb[0:1, mt * 128:(mt + 1) * 128],
                        onesn_sb[0:1, n0:n0 + nw],
                        start=False, stop=True)
                nc.scalar.activation(qk_sb[:, mt, n0:n0 + nw], qp[0:128, 0:nw],
                                     mybir.ActivationFunctionType.Copy)

            def v_group(pool, ct, n0, nw):
                qp = pool.tile([128, 512], F32, tag="av", name="vg")
                for kt in range(4):
                    nc.tensor.matmul(
                        qp[0:128, 0:nw],
                        wqkvt_sb[:, kt, 512 + ct * 128:512 + (ct + 1) * 128],
                        x_sb[:, kt, n0:n0 + nw],
                        start=(kt == 0), stop=False)
                nc.tensor.matmul(
                    qp[0:128, 0:nw],
                    bvrow_sb[0:1, ct * 128:(ct + 1) * 128],
                    onesn_sb[0:1, n0:n0 + nw],
                    start=False, stop=True)
                nc.scalar.activation(v_sb[:, ct, n0:n0 + nw], qp[0:128, 0:nw],
                                     mybir.ActivationFunctionType.Copy)

            def vt_dma(ct, mi):
                m0, mw = MT[mi]
                nc.sync.dma_start_transpose(vtb_sb[0:mw, mi, :],
                                            v_sb[:, ct, m0:m0 + mw])

            def vt12_group(pool):
                # last m-chunk is 64 rows - xbar DMA transpose needs 128 -
                # so compute v^T for it directly with an x-stationary matmul
                # (all 8 heads at once) and evacuate straight to fp8
                vp = pool.tile([64, 512], F32, tag="vt12", name="vt12")
                for kt in range(4):
                    nc.tensor.matmul(
                        vp[0:64, 0:512],
                        x_sb[:, kt, 1536:1600],
                        wqkvt_sb[:, kt, 512:1024],
                        start=(kt == 0), stop=False)
                nc.tensor.matmul(
                    vp[0:64, 0:512],
                    onesn_sb[0:1, 1536:1600],
                    bvrow_sb[0:1, :],
                    start=False, stop=True)
                nc.scalar.activation(
                    vt_sb[0:64, 12, :, 0:HD],
                    vp[0:64, 0:512].rearrange("p (h d) -> p h d", h=NH),
                    mybir.ActivationFunctionType.Copy)

            def vt_conv(ct, g):
                # convert a group of 4 transposed chunks to fp8 in one DVE op
                nc.vector.tensor_copy(
                    out=vt_sb[0:128, 4 * g:4 * g + 4, 2 * ct:2 * ct + 2, 0:HD],
                    in_=vtb_sb[0:128, 4 * g:4 * g + 4, :].rearrange(
                        "p m (h d) -> p m h d", h=2))

            # depthwise 3x3 conv taps (DVE, bf16)
            def dwconv_ops():
                for ct in range(4):
                    def center(ct=ct):
                        nc.vector.tensor_scalar(
                            out=pe_sb[:, ct], in0=v4[:, ct],
                            scalar1=wpe_sb[:, ct * 9 + 4:ct * 9 + 5],
                            scalar2=bpe_sb[:, ct:ct + 1],
                            op0=ALU.mult, op1=ALU.add)
                    yield center
                    for t in range(9):
                        dy, dx = t // 3 - 1, t % 3 - 1
                        if dy == 0 and dx == 0:
                            continue

                        def tap(ct=ct, t=t, dy=dy, dx=dx):
                            ys, ye = max(0, -dy), H - max(0, dy)
                            xs, xe = max(0, -dx), W - max(0, dx)
                            acc = pe_sb[:, ct, ys:ye, xs:xe]
                            nc.vector.scalar_tensor_tensor(
                                out=acc,
                                in0=v4[:, ct, ys + dy:ye + dy, xs + dx:xe + dx],
                                scalar=wpe_sb[:, ct * 9 + t:ct * 9 + t + 1],
                                in1=acc, op0=ALU.mult, op1=ALU.add)
                        yield tap

            # ---------------- phase 1: upfront qkv prefix ----------------
            # q/k for heads 0-3 (tiles 0 and 2) and v ct0 (heads 0,1),
            # which pair-0 sweeps need. The rest drips into the attention
            # pipeline below.
            with tc.tile_pool(name="ps1", bufs=4, space="PSUM") as ps1:
                for mt in (0, 2):
                    for (n0, nw) in NB:
                        qk_group(ps1, mt, n0, nw)
                for (n0, nw) in NB:
                    v_group(ps1, 0, n0, nw)
                vt12_group(ps1)
            for mi in range(12):
                vt_dma(0, mi)
            for g in range(3):
                vt_conv(0, g)

            # ---------------- phase 2: attention sweeps ----------------
            import collections as _c

            # PE-work drip queue (v/qk groups during early pairs, then proj)
            pe_drip = _c.deque()
            # DVE-work drip queue (dwconv taps)
            dve_drip = _c.deque(dwconv_ops())
            # per-pair prefetch drips, consumed during that pair's sweeps
            pair_drip = {1: _c.deque(), 2: _c.deque(), 3: _c.deque()}
            for nb_i, (n0d, nwd) in enumerate(NB):
                pair_drip[1].append(lambda n0=n0d, nw=nwd: v_group(psav, 1, n0, nw))
            for mi_d in range(12):
                pair_drip[1].append(lambda mi=mi_d: vt_dma(1, mi))
            for g_d in range(3):
                pair_drip[1].append(lambda g=g_d: vt_conv(1, g))
            for mt_d in (1, 3):
                for (n0d, nwd) in NB:
                    pair_drip[2].append(lambda mt=mt_d, n0=n0d, nw=nwd: qk_group(psav, mt, n0, nw))
            for (n0d, nwd) in NB:
                pair_drip[2].append(lambda n0=n0d, nw=nwd: v_group(psav, 2, n0, nw))
            for mi_d in range(12):
                pair_drip[2].append(lambda mi=mi_d: vt_dma(2, mi))
            for g_d in range(3):
                pair_drip[2].append(lambda g=g_d: vt_conv(2, g))
            for (n0d, nwd) in NB:
                pair_drip[3].append(lambda n0=n0d, nw=nwd: v_group(psav, 3, n0, nw))
            for mi_d in range(12):
                pair_drip[3].append(lambda mi=mi_d: vt_dma(3, mi))
            for g_d in range(3):
                pair_drip[3].append(lambda g=g_d: vt_conv(3, g))

            with tc.tile_pool(name="ps_s", bufs=2, space="PSUM") as pss, \
                 tc.tile_pool(name="ps_av", bufs=4, space="PSUM") as psav, \
                 tc.tile_pool(name="expp", bufs=3) as expp, \
                 tc.tile_pool(name="nrm", bufs=4) as nrm, \
                 tc.tile_pool(name="outp", bufs=3) as outp:

                def proj_ops(nbi):
                    n0, nw = NB[nbi]
                    for ot in range(4):
                        pp = [None]
                        for kt in range(4):
                            def mm(ot=ot, kt=kt, pp=pp):
                                if kt == 0:
                                    pp[0] = psav.tile([128, 512], F32, tag="av", name="pp")
                                nc.tensor.matmul(
                                    pp[0][0:128, 0:nw],
                                    wprojt_sb[:, kt, ot * 128:(ot + 1) * 128],
                                    attn_sb[:, kt, n0:n0 + nw],
                                    start=(kt == 0), stop=False)
                            yield mm

                        def evac(ot=ot, pp=pp):
                            # proj bias via K=1 ones-row matmul, then ScalarE
                            # evacuates PSUM; Pool (no PSUM access) adds the
                            # SBUF-resident pe term
                            nc.tensor.matmul(
                                pp[0][0:128, 0:nw],
                                bproj_sb[0:1, ot * 128:(ot + 1) * 128],
                                onesn_sb[0:1, n0:n0 + nw],
                                start=False, stop=True)
                            ob = outp.tile([128, 512], F32, tag="ob")
                            nc.scalar.activation(ob[0:128, 0:nw], pp[0][0:128, 0:nw],
                                                 mybir.ActivationFunctionType.Copy)
                            nc.gpsimd.tensor_tensor(
                                out=ob[0:128, 0:nw], in0=ob[0:128, 0:nw],
                                in1=pe3[:, ot, n0:n0 + nw], op=ALU.add)
                            nc.sync.dma_start(out_dr[:, ot, n0:n0 + nw],
                                              ob[0:128, 0:nw])
                        yield evac

                def normalize_a(avs, nw):
                    # Evacuate av+denominator rows to SBUF bf16 (frees the
                    # PSUM accumulator), then build a [64, nw] broadcast of
                    # 1/D entirely with DMAs: [32,16]-reshaped reciprocal
                    # (iterative divide paid on 16 elems/partition), then
                    # log2-doubling row copies. No engine time beyond the
                    # evac copy and the tiny reciprocal.
                    nws = max(nw // 32, 1)
                    out = []
                    for j in range(2):
                        av_s = nrm.tile([HD + 1, 512], F32, tag="avs")
                        dsplit = nrm.tile([32, 16], F32, tag="dsplit")
                        rsplit = nrm.tile([32, 16], F32, tag="rsplit")
                        rc = nrm.tile([1, 512], F32, tag="rc")
                        rbs = nrm.tile([64, 512], F32, tag="rbs")
                        if j == 0:
                            nc.vector.tensor_copy(av_s[0:HD + 1, 0:nw],
                                                  avs[j][0:HD + 1, 0:nw])
                        else:
                            nc.scalar.activation(av_s[0:HD + 1, 0:nw],
                                                 avs[j][0:HD + 1, 0:nw],
                                                 mybir.ActivationFunctionType.Copy)
                        nc.sync.dma_start(dsplit[0:32, 0:nws], av_s[HD:HD + 1, 0:nw])
                        nc.vector.reciprocal(rsplit[0:32, 0:nws], dsplit[0:32, 0:nws])
                        nc.sync.dma_start(rc[0:1, 0:nw], rsplit[0:32, 0:nws])
                        nc.gpsimd.partition_broadcast(rbs[0:64, 0:nw], rc[0:1, 0:nw])
                        out.append((av_s, rbs))
                    return out

                def normalize_b(p, n0, nw, norm_st):
                    for j in range(2):
                        av_s, rbs = norm_st[j]
                        nc.vector.scalar_tensor_tensor(
                            out=attn_sb[j * 64:j * 64 + 64, p, n0:n0 + nw],
                            in0=av_s[0:HD, 0:nw], scalar=1.0,
                            in1=rbs[0:64, 0:nw],
                            op0=ALU.bypass, op1=ALU.mult)

                def drip_one(p):
                    for q in range(1, min(p + 1, 3) + 1):
                        if pair_drip[q]:
                            pair_drip[q].popleft()()
                            return
                    if pe_drip:
                        pe_drip.popleft()()

                pend_av = [None]
                pend_norm = [None]

                def fire_pend():
                    if pend_av[0] is not None:
                        pend_av[0]()
                        pend_av[0] = None

                def fire_norm():
                    if pend_norm[0] is not None:
                        pend_norm[0]()
                        pend_norm[0] = None

                for p in range(4):
                    tq, pb = p // 2, (p % 2) * 64
                    for nbi in range(4):
                        n0, nw = NB[nbi]
                        avs = (psav.tile([128, 512], F32, tag="av", name="av0"),
                               psav.tile([128, 512], F32, tag="av", name="av1"))
                        es = None
                        for mi, (m0, mw) in enumerate(MT):
                            if mi % 2 == 0:
                                es = expp.tile([128, 2, 2, 512], FP8, tag="es")
                            sp = pss.tile([128, 1024], F32, tag="sp")
                            sp3 = sp[:].rearrange("p (j n) -> p j n", j=2)
                            for j in range(2):
                                nc.tensor.matmul(
                                    sp[0:mw, j * 512:j * 512 + nw],
                                    qk_sb[pb + 32 * j:pb + 32 * j + 32, 2 + tq, m0:m0 + mw],
                                    qk_sb[pb + 32 * j:pb + 32 * j + 32, tq, n0:n0 + nw],
                                    start=True, stop=True,
                                    tile_position=(pb + 32 * j, 0))
                            eng = EXP_ENG[mi]
                            dst = es[0:mw, mi % 2, :, 0:nw]
                            if eng == 'S':
                                nc.scalar.activation(dst, sp3[0:mw, :, 0:nw],
                                                     EXP, scale=SCALE)
                            else:
                                e = nc.vector if eng == 'D' else nc.gpsimd
                                e.tensor_scalar(
                                    out=dst.bitcast(I8), in0=sp3[0:mw, :, 0:nw],
                                    scalar1=SCH_A, scalar2=SCH_B,
                                    op0=ALU.mult, op1=ALU.add)
                            fire_pend()
                            if mi == NORM_FIRE_MI:
                                fire_norm()
                            drip_one(p)
                            if mi in (4, 8, 12) and dve_drip:
                                dve_drip.popleft()()
                            if mi % 2 == 1:
                                def av_pair(es=es, mi=mi, nw=nw, avs=avs, p=p):
                                    for j in range(2):
                                        nc.tensor.matmul(
                                            avs[j][0:HD + 1, 0:nw],
                                            vt_sb[0:128, mi - 1:mi + 1, 2 * p + j, 0:HD + 1],
                                            es[0:128, :, j, 0:nw],
                                            start=(mi == 1), stop=False,
                                            perf_mode=DRM)
                                pend_av[0] = av_pair
                            elif mi == 12:
                                def av_last(es=es, nw=nw, avs=avs, p=p):
                                    for j in range(2):
                                        nc.tensor.matmul(
                                            avs[j][0:HD + 1, 0:nw],
                                            vt_sb[0:64, 12, 2 * p + j, 0:HD + 1],
                                            es[0:64, 0, j, 0:nw],
                                            start=False, stop=True)
                                pend_av[0] = av_last
                        fire_pend()
                        norm_st = normalize_a(avs, nw)

                        def norm_b(p=p, n0=n0, nw=nw, norm_st=norm_st, nbi=nbi):
                            normalize_b(p, n0, nw, norm_st)
                            if p == 3:
                                pe_drip.extend(proj_ops(nbi))
                        pend_norm[0] = norm_b
                # drain
                fire_norm()
                while pe_drip:
                    pe_drip.popleft()()
                while dve_drip:
                    dve_drip.popleft()()

    nc.compile()
    return nc


def _prep(Wqkv, bqkv, Wproj, bproj, Wpe, bpe):
    WqkvT = np.ascontiguousarray(Wqkv.T)            # [512, 1024]
    wqkvt_h = np.ascontiguousarray(
        WqkvT.reshape(4, 128, 1024).transpose(1, 0, 2).reshape(128, 4096)
    ).astype(ml_dtypes.bfloat16)
    WprojT = np.ascontiguousarray(Wproj.T)          # [512, 512]
    wprojt_h = np.ascontiguousarray(
        WprojT.reshape(4, 128, 512).transpose(1, 0, 2).reshape(128, 2048)
    ).astype(ml_dtypes.bfloat16)
    bqrow_h = np.ascontiguousarray(bqkv[0:256].reshape(1, 256)).astype(ml_dtypes.bfloat16)
    bvrow_h = np.ascontiguousarray(bqkv[512:1024].reshape(1, 512)).astype(ml_dtypes.bfloat16)
    bproj_h = np.ascontiguousarray(bproj.reshape(1, 512)).astype(ml_dtypes.bfloat16)
    bpe_h = np.ascontiguousarray(bpe.reshape(4, 128).T)
    wpe_h = np.ascontiguousarray(
        Wpe.reshape(512, 9).reshape(4, 128, 9).transpose(1, 0, 2).reshape(128, 36))
    return dict(wqkvt=wqkvt_h, wprojt=wprojt_h, bqrow=bqrow_h, bvrow=bvrow_h,
                bproj=bproj_h, bpe=bpe_h, wpe=wpe_h,
                ones8=np.ones((128, NH), dtype=ml_dtypes.float8_e4m3))


def kernel(x, Wqkv, bqkv, Wproj, bproj, Wpe, bpe, _trace=False, _trace_kwargs=None):
    x = np.asarray(x, dtype=np.float32)
    Wqkv = np.asarray(Wqkv, dtype=np.float32)
    bqkv = np.asarray(bqkv, dtype=np.float32)
    Wproj = np.asarray(Wproj, dtype=np.float32)
    bproj = np.asarray(bproj, dtype=np.float32)
    Wpe = np.asarray(Wpe, dtype=np.float32)
    bpe = np.asarray(bpe, dtype=np.float32)
    B = x.shape[0]
    if "nc" not in _CACHE:
        _CACHE["nc"] = build()
    nc = _CACHE["nc"]
    shared = _prep(Wqkv, bqkv, Wproj, bproj, Wpe, bpe)
    xb = np.ascontiguousarray(x.reshape(B, C, N)).astype(ml_dtypes.bfloat16)
    in_maps = [dict(shared, x=xb[b]) for b in range(B)]
    res = run_bass_kernel_spmd(nc, in_maps, core_ids=list(range(8)),
                               trace=_trace, **(_trace_kwargs or {}))
    out = np.stack([res.results[b]["out"] for b in range(B)])
    kernel.last_result = res
    return out.reshape(B, C, H, W).astype(np.float32)


# revision 37
# speedup vs baseline: 1.1424x; 1.1424x over previous
"""Trainium2 Bass kernel for nn_Attention_16707422781936.

Data-parallel over batch: B=8 -> one batch element per NeuronCore (8 cores).
Per core: qkv 1x1-conv GEMM, 8-head softmax attention over N=1600 tokens,
proj GEMM, depthwise 3x3 positional-encoding conv, summed output.

Optimizations over the v1 kernel:
 - softmax exp split across ScalarE (exact, fp8 out) and DVE/Pool
   (Schraudolph int8 bit-trick producing fp8e4m3 directly)
 - AV matmul in fp8e4 DoubleRow mode (256-deep contraction per instr)
 - scores matmuls cycle PE row-bands (mi parity) -> 4-band concurrency
 - v computed once (channel-major); token-major copy via DMA transpose
 - v bias folded into bpe (host) so v is bias-free on device
 - dwconv in bf16 on DVE (2x mode); evacuations moved to Pool engine
 - softmax normalize uses a 0-stride partition-broadcast AP (no
   gpsimd partition_broadcast op)
"""
import math
import sys

sys.path.insert(0, "/opt/trn_rl_repo")

import ml_dtypes
import numpy as np

import concourse.bass as bass
import concourse.mybir as mybir
import concourse.tile as tile
from concourse import bacc
from concourse.bass_utils import run_bass_kernel_spmd

F32 = mybir.dt.float32
F32R = mybir.dt.float32r
BF16 = mybir.dt.bfloat16
FP8 = mybir.dt.float8e4
I8 = mybir.dt.int8
ALU = mybir.AluOpType
EXP = mybir.ActivationFunctionType.Exp
DRM = mybir.MatmulPerfMode.DoubleRow

C = 512          # channels
N = 1600         # tokens (40*40)
H = W = 40
NH = 8           # heads
KD = 32          # key dim
HD = 64          # head dim (v)
SCALE = KD ** -0.5

# Schraudolph fast-exp constants for fp8e4m3 bit pattern:
# bits = round(s * SCALE * 8/ln2 + (7*8 - C_ADJ)); bits viewed as fp8e4m3
# approximate exp(s * SCALE).
SCH_C = -0.1
SCH_A = 8.0 * SCALE / math.log(2.0)
SCH_B = 7.0 * 8.0 - SCH_C

# n blocks (psum-bank sized) and m tiles (partition sized)
NB = [(0, 512), (512, 512), (1024, 512), (1536, 64)]
MT = [(i * 128, min(128, N - i * 128)) for i in range(13)]
# exp chunk engine per mi: S=ScalarE exact, D=DVE schraudolph
# (Pool/gpsimd cannot read PSUM, so it gets no exp chunks)
EXP_ENG = ['S', 'D', 'S', 'D', 'S', 'D', 'S', 'D', 'S', 'D', 'S', 'D', 'S']
VT_W = 72        # padded vt row width (65 used) so DR k-pair stride % 16 == 0
NORM_FIRE_MI = 4   # sweep step at which the previous sweep's deferred normalize fires

_CACHE = {}


def build():
    nc = bacc.Bacc("TRN2", target_bir_lowering=False, debug=False,
                   enable_asserts=False)

    x_d = nc.dram_tensor("x", [C, N], BF16, kind="ExternalInput").ap()
    wqkvt_d = nc.dram_tensor("wqkvt", [128, 4 * 1024], BF16, kind="ExternalInput").ap()
    wprojt_d = nc.dram_tensor("wprojt", [128, 4 * 512], BF16, kind="ExternalInput").ap()
    bqrow_d = nc.dram_tensor("bqrow", [1, 256], BF16, kind="ExternalInput").ap()
    bvrow_d = nc.dram_tensor("bvrow", [1, 512], BF16, kind="ExternalInput").ap()
    bproj_d = nc.dram_tensor("bproj", [1, 512], BF16, kind="ExternalInput").ap()
    wpediag_d = nc.dram_tensor("wpediag", [128, 36 * 128], BF16, kind="ExternalInput").ap()
    bperow_d = nc.dram_tensor("bperow", [1, 512], BF16, kind="ExternalInput").ap()
    ones_d = nc.dram_tensor("ones8", [128, NH], FP8, kind="ExternalInput").ap()
    out_d = nc.dram_tensor("out", [C, N], F32, kind="ExternalOutput").ap()

    with nc.allow_low_precision(reason="bf16/fp8 attention within 2e-2 tolerance"), \
         tile.TileContext(nc) as tc:
        with tc.tile_pool(name="persist", bufs=1) as per:
            x_sb = per.tile([128, 4, N], BF16, tag="x")
            qk_sb = per.tile([128, 4, N], BF16, tag="qk")   # q(h0-3),q(h4-7),k(h0-3),k(h4-7)
            v_sb = per.tile([128, 4, N], BF16, tag="v")     # v channel-major, no bias
            vtb_sb = per.tile([128, 13, 128], BF16, tag="vtb")  # DMA-transpose staging
            vt_sb = per.tile([128, 13, NH, VT_W], FP8, tag="vt")  # v^T fp8 + ones col
            attn_sb = per.tile([128, 4, N], BF16, tag="attn")
            pe_sb = per.tile([128, 4, H, W], BF16, tag="pe")
            wqkvt_sb = per.tile([128, 4, 1024], BF16, tag="wqkvt")
            wprojt_sb = per.tile([128, 4, 512], BF16, tag="wprojt")
            bqrow_sb = per.tile([1, 256], BF16, tag="bqrow")
            bvrow_sb = per.tile([1, 512], BF16, tag="bvrow")
            bproj_sb = per.tile([1, 512], BF16, tag="bproj")
            wpediag_sb = per.tile([128, 36, 128], BF16, tag="wpediag")
            bperow_sb = per.tile([1, 512], BF16, tag="bperow")

            nc.sync.dma_start(wqkvt_sb[:], wqkvt_d.rearrange("p (t o) -> p t o", t=4))
            nc.sync.dma_start(wprojt_sb[:], wprojt_d.rearrange("p (t o) -> p t o", t=4))
            nc.sync.dma_start(bqrow_sb[:], bqrow_d)
            nc.sync.dma_start(bvrow_sb[:], bvrow_d)
            nc.sync.dma_start(bproj_sb[:], bproj_d)
            nc.sync.dma_start(wpediag_sb[:],
                              wpediag_d.rearrange("p (t o) -> p t o", t=36))
            nc.sync.dma_start(bperow_sb[:], bperow_d)
            x_dr = x_d.rearrange("(t p) n -> p t n", p=128)
            for kt in range(4):
                nc.sync.dma_start(x_sb[:, kt], x_dr[:, kt])
            for mi in range(13):
                nc.sync.dma_start(vt_sb[:, mi, :, HD:HD + 1],
                                  ones_d.rearrange("p (h o) -> p h o", o=1))
            # [1, N] bf16 ones row: K=1 matmul adds the q bias into the
            # qkv PSUM accumulation (k bias cancels in softmax; v bias is
            # folded into bproj/bpe on the host)
            onesn_sb = per.tile([1, N], BF16, tag="onesn")
            nc.gpsimd.memset(onesn_sb[:], 1.0)

            v4 = v_sb[:].rearrange("p t (h w) -> p t h w", h=H)
            pe3 = pe_sb[:].rearrange("p t h w -> p t (h w)")
            out_dr = out_d.rearrange("(t p) n -> p t n", p=128)

            # ---------- building blocks ----------
            def qk_group(pool, mt, n0, nw):
                is_q = mt < 2
                qp = pool.tile([128, 512], F32, tag="av", name="qg")
                for kt in range(4):
                    nc.tensor.matmul(
                        qp[0:128, 0:nw],
                        wqkvt_sb[:, kt, mt * 128:(mt + 1) * 128],
                        x_sb[:, kt, n0:n0 + nw],
                        start=(kt == 0), stop=(kt == 3 and not is_q))
                if is_q:
                    nc.tensor.matmul(
                        qp[0:128, 0:nw],
                        bqrow_sb[0:1, mt * 128:(mt + 1) * 128],
                        onesn_sb[0:1, n0:n0 + nw],
                        start=False, stop=True)
                nc.scalar.activation(qk_sb[:, mt, n0:n0 + nw], qp[0:128, 0:nw],
                                     mybir.ActivationFunctionType.Copy)

            def v_group(pool, ct, n0, nw):
                qp = pool.tile([128, 512], F32, tag="av", name="vg")
                for kt in range(4):
                    nc.tensor.matmul(
                        qp[0:128, 0:nw],
                        wqkvt_sb[:, kt, 512 + ct * 128:512 + (ct + 1) * 128],
                        x_sb[:, kt, n0:n0 + nw],
                        start=(kt == 0), stop=False)
                nc.tensor.matmul(
                    qp[0:128, 0:nw],
                    bvrow_sb[0:1, ct * 128:(ct + 1) * 128],
                    onesn_sb[0:1, n0:n0 + nw],
                    start=False, stop=True)
                nc.scalar.activation(v_sb[:, ct, n0:n0 + nw], qp[0:128, 0:nw],
                                     mybir.ActivationFunctionType.Copy)

            def vt_dma(ct, mi):
                m0, mw = MT[mi]
                nc.sync.dma_start_transpose(vtb_sb[0:mw, mi, :],
                                            v_sb[:, ct, m0:m0 + mw])

            def vt12_group(pool):
                # last m-chunk is 64 rows - xbar DMA transpose needs 128 -
                # so compute v^T for it directly with an x-stationary matmul
                # (all 8 heads at once) and evacuate straight to fp8
                vp = pool.tile([64, 512], F32, tag="vt12", name="vt12")
                for kt in range(4):
                    nc.tensor.matmul(
                        vp[0:64, 0:512],
                        x_sb[:, kt, 1536:1600],
                        wqkvt_sb[:, kt, 512:1024],
                        start=(kt == 0), stop=False)
                nc.tensor.matmul(
                    vp[0:64, 0:512],
                    onesn_sb[0:1, 1536:1600],
                    bvrow_sb[0:1, :],
                    start=False, stop=True)
                nc.scalar.activation(
                    vt_sb[0:64, 12, :, 0:HD],
                    vp[0:64, 0:512].rearrange("p (h d) -> p h d", h=NH),
                    mybir.ActivationFunctionType.Copy)

            def vt_conv(ct, g):
                # convert a group of 4 transposed chunks to fp8 in one DVE op
                nc.vector.tensor_copy(
                    out=vt_sb[0:128, 4 * g:4 * g + 4, 2 * ct:2 * ct + 2, 0:HD],
                    in_=vtb_sb[0:128, 4 * g:4 * g + 4, :].rearrange(
                        "p m (h d) -> p m h d", h=2))

            # depthwise 3x3 conv as diag-weight matmuls on the PE, in
            # 10-image-row blocks (400 pixels fits one PSUM bank). Tap
            # order: center first (full region, resets PSUM), then edge
            # taps accumulate sub-regions, then a K=1 bias matmul.
            PE_TAPS = [4, 0, 1, 2, 3, 5, 6, 7, 8]

            def pe_ops(ct, rb):
                r0, r1 = rb * 10, rb * 10 + 10
                pp = [None]

                def taps(lo, hi, ct=ct, r0=r0, r1=r1, pp=pp):
                    if lo == 0:
                        pp[0] = psav.tile([128, 512], F32, tag="av", name="pe")
                    p3 = pp[0][0:128, 0:400].rearrange("p (h w) -> p h w", h=10)
                    for t in PE_TAPS[lo:hi]:
                        dy, dx = t // 3 - 1, t % 3 - 1
                        ys = max(r0, -dy)
                        ye = min(r1, H - max(0, dy))
                        xs, xe = max(0, -dx), W - max(0, dx)
                        nc.tensor.matmul(
                            p3[:, ys - r0:ye - r0, xs:xe],
                            wpediag_sb[:, ct * 9 + t, :],
                            v4[:, ct, ys + dy:ye + dy, xs + dx:xe + dx],
                            start=(t == 4), stop=False)
                    if hi == 9:
                        nc.tensor.matmul(
                            pp[0][0:128, 0:400],
                            bperow_sb[0:1, ct * 128:(ct + 1) * 128],
                            onesn_sb[0:1, 0:400],
                            start=False, stop=True)

                def evac(ct=ct, rb=rb, r0=r0, r1=r1, pp=pp):
                    nc.scalar.activation(
                        pe_sb[:, ct, r0:r1, :],
                        pp[0][0:128, 0:400].rearrange("p (h w) -> p h w", h=10),
                        mybir.ActivationFunctionType.Copy)

                yield lambda: taps(0, 5)
                yield lambda: taps(5, 9)
                yield evac

            # ---------------- phase 1: upfront qkv prefix ----------------
            # q/k for heads 0-3 (tiles 0 and 2) and v ct0 (heads 0,1),
            # which pair-0 sweeps need. The rest drips into the attention
            # pipeline below.
            with tc.tile_pool(name="ps1", bufs=4, space="PSUM") as ps1:
                for mt in (0, 2):
                    for (n0, nw) in NB:
                        qk_group(ps1, mt, n0, nw)
                for (n0, nw) in NB:
                    v_group(ps1, 0, n0, nw)
                vt12_group(ps1)
            for mi in range(12):
                vt_dma(0, mi)
            for g in range(3):
                vt_conv(0, g)

            # ---------------- phase 2: attention sweeps ----------------
            import collections as _c

            # PE-work drip queue (v/qk groups during early pairs, then proj)
            pe_drip = _c.deque()
            # per-pair prefetch drips, consumed during that pair's sweeps
            pair_drip = {1: _c.deque(), 2: _c.deque(), 3: _c.deque()}
            for nb_i, (n0d, nwd) in enumerate(NB):
                pair_drip[1].append(lambda n0=n0d, nw=nwd: v_group(psav, 1, n0, nw))
            for mi_d in range(12):
                pair_drip[1].append(lambda mi=mi_d: vt_dma(1, mi))
            for g_d in range(3):
                pair_drip[1].append(lambda g=g_d: vt_conv(1, g))
            for rb_d in range(4):
                pair_drip[1].extend(pe_ops(0, rb_d))
            for mt_d in (1, 3):
                for (n0d, nwd) in NB:
                    pair_drip[2].append(lambda mt=mt_d, n0=n0d, nw=nwd: qk_group(psav, mt, n0, nw))
            for (n0d, nwd) in NB:
                pair_drip[2].append(lambda n0=n0d, nw=nwd: v_group(psav, 2, n0, nw))
            for mi_d in range(12):
                pair_drip[2].append(lambda mi=mi_d: vt_dma(2, mi))
            for g_d in range(3):
                pair_drip[2].append(lambda g=g_d: vt_conv(2, g))
            for rb_d in range(4):
                pair_drip[2].extend(pe_ops(1, rb_d))
            for (n0d, nwd) in NB:
                pair_drip[3].append(lambda n0=n0d, nw=nwd: v_group(psav, 3, n0, nw))
            for mi_d in range(12):
                pair_drip[3].append(lambda mi=mi_d: vt_dma(3, mi))
            for g_d in range(3):
                pair_drip[3].append(lambda g=g_d: vt_conv(3, g))
            for ct_d in (2, 3):
                for rb_d in range(4):
                    pair_drip[3].extend(pe_ops(ct_d, rb_d))

            with tc.tile_pool(name="ps_s", bufs=2, space="PSUM") as pss, \
                 tc.tile_pool(name="ps_av", bufs=4, space="PSUM") as psav, \
                 tc.tile_pool(name="expp", bufs=3) as expp, \
                 tc.tile_pool(name="nrm", bufs=4) as nrm, \
                 tc.tile_pool(name="outp", bufs=3) as outp:

                def proj_ops(nbi):
                    n0, nw = NB[nbi]
                    for ot in range(4):
                        pp = [None]
                        for kt in range(4):
                            def mm(ot=ot, kt=kt, pp=pp):
                                if kt == 0:
                                    pp[0] = psav.tile([128, 512], F32, tag="av", name="pp")
                                nc.tensor.matmul(
                                    pp[0][0:128, 0:nw],
                                    wprojt_sb[:, kt, ot * 128:(ot + 1) * 128],
                                    attn_sb[:, kt, n0:n0 + nw],
                                    start=(kt == 0), stop=False)
                            yield mm

                        def evac(ot=ot, pp=pp):
                            # proj bias via K=1 ones-row matmul, then ScalarE
                            # evacuates PSUM; Pool (no PSUM access) adds the
                            # SBUF-resident pe term
                            nc.tensor.matmul(
                                pp[0][0:128, 0:nw],
                                bproj_sb[0:1, ot * 128:(ot + 1) * 128],
                                onesn_sb[0:1, n0:n0 + nw],
                                start=False, stop=True)
                            ob = outp.tile([128, 512], F32, tag="ob")
                            nc.scalar.activation(ob[0:128, 0:nw], pp[0][0:128, 0:nw],
                                                 mybir.ActivationFunctionType.Copy)
                            nc.gpsimd.tensor_tensor(
                                out=ob[0:128, 0:nw], in0=ob[0:128, 0:nw],
                                in1=pe3[:, ot, n0:n0 + nw], op=ALU.add)
                            nc.sync.dma_start(out_dr[:, ot, n0:n0 + nw],
                                              ob[0:128, 0:nw])
                        yield evac

                def normalize_a(avs, nw):
                    # Evacuate av+denominator rows to SBUF bf16 (frees the
                    # PSUM accumulator), then build a [64, nw] broadcast of
                    # 1/D entirely with DMAs: [32,16]-reshaped reciprocal
                    # (iterative divide paid on 16 elems/partition), then
                    # log2-doubling row copies. No engine time beyond the
                    # evac copy and the tiny reciprocal.
                    nws = max(nw // 32, 1)
                    out = []
                    for j in range(2):
                        av_s = nrm.tile([HD + 1, 512], F32, tag="avs")
                        dsplit = nrm.tile([32, 16], F32, tag="dsplit")
                        rsplit = nrm.tile([32, 16], F32, tag="rsplit")
                        rc = nrm.tile([1, 512], F32, tag="rc")
                        rbs = nrm.tile([64, 512], F32, tag="rbs")
                        if j == 0:
                            nc.vector.tensor_copy(av_s[0:HD + 1, 0:nw],
                                                  avs[j][0:HD + 1, 0:nw])
                        else:
                            nc.scalar.activation(av_s[0:HD + 1, 0:nw],
                                                 avs[j][0:HD + 1, 0:nw],
                                                 mybir.ActivationFunctionType.Copy)
                        nc.sync.dma_start(dsplit[0:32, 0:nws], av_s[HD:HD + 1, 0:nw])
                        nc.vector.reciprocal(rsplit[0:32, 0:nws], dsplit[0:32, 0:nws])
                        nc.sync.dma_start(rc[0:1, 0:nw], rsplit[0:32, 0:nws])
                        nc.gpsimd.partition_broadcast(rbs[0:64, 0:nw], rc[0:1, 0:nw])
                        out.append((av_s, rbs))
                    return out

                def normalize_b(p, n0, nw, norm_st):
                    for j in range(2):
                        av_s, rbs = norm_st[j]
                        nc.vector.scalar_tensor_tensor(
                            out=attn_sb[j * 64:j * 64 + 64, p, n0:n0 + nw],
                            in0=av_s[0:HD, 0:nw], scalar=1.0,
                            in1=rbs[0:64, 0:nw],
                            op0=ALU.bypass, op1=ALU.mult)

                def drip_one(p):
                    for q in range(1, min(p + 1, 3) + 1):
                        if pair_drip[q]:
                            pair_drip[q].popleft()()
                            return
                    if pe_drip:
                        pe_drip.popleft()()

                pend_av = [None]
                pend_norm = [None]

                def fire_pend():
                    if pend_av[0] is not None:
                        pend_av[0]()
                        pend_av[0] = None

                def fire_norm():
                    if pend_norm[0] is not None:
                        pend_norm[0]()
                        pend_norm[0] = None

                for p in range(4):
                    tq, pb = p // 2, (p % 2) * 64
                    for nbi in range(4):
                        n0, nw = NB[nbi]
                        avs = (psav.tile([128, 512], F32, tag="av", name="av0"),
                               psav.tile([128, 512], F32, tag="av", name="av1"))
                        es = None
                        for mi, (m0, mw) in enumerate(MT):
                            if mi % 2 == 0:
                                es = expp.tile([128, 2, 2, 512], FP8, tag="es")
                            sp = pss.tile([128, 1024], F32, tag="sp")
                            sp3 = sp[:].rearrange("p (j n) -> p j n", j=2)
                            for j in range(2):
                                nc.tensor.matmul(
                                    sp[0:mw, j * 512:j * 512 + nw],
                                    qk_sb[pb + 32 * j:pb + 32 * j + 32, 2 + tq, m0:m0 + mw],
                                    qk_sb[pb + 32 * j:pb + 32 * j + 32, tq, n0:n0 + nw],
                                    start=True, stop=True,
                                    tile_position=(pb + 32 * j, 0))
                            eng = EXP_ENG[mi]
                            dst = es[0:mw, mi % 2, :, 0:nw]
                            if eng == 'S':
                                nc.scalar.activation(dst, sp3[0:mw, :, 0:nw],
                                                     EXP, scale=SCALE)
                            else:
                                e = nc.vector if eng == 'D' else nc.gpsimd
                                e.tensor_scalar(
                                    out=dst.bitcast(I8), in0=sp3[0:mw, :, 0:nw],
                                    scalar1=SCH_A, scalar2=SCH_B,
                                    op0=ALU.mult, op1=ALU.add)
                            fire_pend()
                            if mi == NORM_FIRE_MI:
                                fire_norm()
                            drip_one(p)
                            if mi % 2 == 1:
                                def av_pair(es=es, mi=mi, nw=nw, avs=avs, p=p):
                                    for j in range(2):
                                        nc.tensor.matmul(
                                            avs[j][0:HD + 1, 0:nw],
                                            vt_sb[0:128, mi - 1:mi + 1, 2 * p + j, 0:HD + 1],
                                            es[0:128, :, j, 0:nw],
                                            start=(mi == 1), stop=False,
                                            perf_mode=DRM)
                                pend_av[0] = av_pair
                            elif mi == 12:
                                def av_last(es=es, nw=nw, avs=avs, p=p):
                                    for j in range(2):
                                        nc.tensor.matmul(
                                            avs[j][0:HD + 1, 0:nw],
                                            vt_sb[0:64, 12, 2 * p + j, 0:HD + 1],
                                            es[0:64, 0, j, 0:nw],
                                            start=False, stop=True)
                                pend_av[0] = av_last
                        fire_pend()
                        norm_st = normalize_a(avs, nw)

                        def norm_b(p=p, n0=n0, nw=nw, norm_st=norm_st, nbi=nbi):
                            normalize_b(p, n0, nw, norm_st)
                            if p == 3:
                                pe_drip.extend(proj_ops(nbi))
                        pend_norm[0] = norm_b
                # drain
                fire_norm()
                while pe_drip:
                    pe_drip.popleft()()

    nc.compile()
    return nc


def _prep(Wqkv, bqkv, Wproj, bproj, Wpe, bpe):
    WqkvT = np.ascontiguousarray(Wqkv.T)            # [512, 1024]
    wqkvt_h = np.ascontiguousarray(
        WqkvT.reshape(4, 128, 1024).transpose(1, 0, 2).reshape(128, 4096)
    ).astype(ml_dtypes.bfloat16)
    WprojT = np.ascontiguousarray(Wproj.T)          # [512, 512]
    wprojt_h = np.ascontiguousarray(
        WprojT.reshape(4, 128, 512).transpose(1, 0, 2).reshape(128, 2048)
    ).astype(ml_dtypes.bfloat16)
    bqrow_h = np.ascontiguousarray(bqkv[0:256].reshape(1, 256)).astype(ml_dtypes.bfloat16)
    bvrow_h = np.ascontiguousarray(bqkv[512:1024].reshape(1, 512)).astype(ml_dtypes.bfloat16)
    bproj_h = np.ascontiguousarray(bproj.reshape(1, 512)).astype(ml_dtypes.bfloat16)
    wpe9 = Wpe.reshape(512, 9)
    wd = np.zeros((128, 36, 128), np.float32)
    idx = np.arange(128)
    for ct in range(4):
        for t in range(9):
            wd[idx, ct * 9 + t, idx] = wpe9[ct * 128:(ct + 1) * 128, t]
    wpediag_h = np.ascontiguousarray(wd.reshape(128, 36 * 128)).astype(ml_dtypes.bfloat16)
    bperow_h = np.ascontiguousarray(bpe.reshape(1, 512)).astype(ml_dtypes.bfloat16)
    return dict(wqkvt=wqkvt_h, wprojt=wprojt_h, bqrow=bqrow_h, bvrow=bvrow_h,
                bproj=bproj_h, wpediag=wpediag_h, bperow=bperow_h,
                ones8=np.ones((128, NH), dtype=ml_dtypes.float8_e4m3))


def kernel(x, Wqkv, bqkv, Wproj, bproj, Wpe, bpe, _trace=False, _trace_kwargs=None):
    x = np.asarray(x, dtype=np.float32)
    Wqkv = np.asarray(Wqkv, dtype=np.float32)
    bqkv = np.asarray(bqkv, dtype=np.float32)
    Wproj = np.asarray(Wproj, dtype=np.float32)
    bproj = np.asarray(bproj, dtype=np.float32)
    Wpe = np.asarray(Wpe, dtype=np.float32)
    bpe = np.asarray(bpe, dtype=np.float32)
    B = x.shape[0]
    if "nc" not in _CACHE:
        _CACHE["nc"] = build()
    nc = _CACHE["nc"]
    shared = _prep(Wqkv, bqkv, Wproj, bproj, Wpe, bpe)
    xb = np.ascontiguousarray(x.reshape(B, C, N)).astype(ml_dtypes.bfloat16)
    in_maps = [dict(shared, x=xb[b]) for b in range(B)]
    res = run_bass_kernel_spmd(nc, in_maps, core_ids=list(range(8)),
                               trace=_trace, **(_trace_kwargs or {}))
    out = np.stack([res.results[b]["out"] for b in range(B)])
    kernel.last_result = res
    return out.reshape(B, C, H, W).astype(np.float32)
